# revision 37
# baseline (speedup 1.0000x reference)
"""Self-contained Trainium2 Bass kernel for nn_Attention_51840255263121.

Full attention block: QKV projection + QK-RMSNorm + RoPE (rotate-half) +
non-causal SDPA + output projection, for B=2, N=2048, C=2048, H=16, D=128.

Sharding: 8 NeuronCores over (batch, head-group): core = b*4 + hg owns batch b
and heads hg*4..hg*4+3 (512 channels). Each core computes its heads' attention
output and a partial output projection over its 512 channels; the host sums the
4 partials per batch and adds the bias.

v1 design notes (vs the f32r baseline at 541 us):
- all matmul inputs bf16 (f32 PSUM accumulation): FWL weight loads, half the
  DMA and SBUF footprint. rmsnorm stats are taken from the f32 PSUM before
  quantization.
- qT/kT planes stay resident in SBUF (no DRAM roundtrip between stages).
- softmax denominators come off the PE: DVE accumulates exp tiles (bf16),
  gpsimd partition_all_reduce + reciprocal_approx_fast + partition_broadcast
  produce the 1/denominator broadcast. This removes 256 F=512 matmuls.
- exp runs on [128,1024] 2-PSUM-bank tiles to amortize ACT overhead.
- output projection is fused into the attention loop (one q-block behind), so
  proj matmuls + output DMA overlap attention compute.
"""

import numpy as np

B, N, C, H, D = 2, 2048, 2048, 16, 128
NCORES = 8
HPC = 4          # heads per core
CS = HPC * D     # 512 channels per core
NT = N // 128    # 16 n-tiles
CT = C // 128    # 16 c-tiles
EPS = 1e-6
NCHUNK = 512     # stage-A xT n-chunk
NQC = 512        # attention q chunk


def build_nc():
    import concourse.bacc as bacc
    import concourse.mybir as mybir
    import concourse.tile as tile
    import concourse.bass_isa as bass_isa
    from concourse.masks import make_identity

    F32 = mybir.dt.float32
    BF16 = mybir.dt.bfloat16
    AF = mybir.ActivationFunctionType
    ALU = mybir.AluOpType

    nc = bacc.Bacc(None, target_bir_lowering=False, debug=False)

    xT = nc.declare_dram_parameter("xT", [C, N], BF16, isOutput=False)
    wT = nc.declare_dram_parameter("wT", [C, 3 * CS], BF16, isOutput=False)
    pwT = nc.declare_dram_parameter("pwT", [CS, C], BF16, isOutput=False)
    cosq = nc.declare_dram_parameter("cosq", [N, D], BF16, isOutput=False)
    sinq = nc.declare_dram_parameter("sinq", [N, D], BF16, isOutput=False)
    cosk = nc.declare_dram_parameter("cosk", [N, D], BF16, isOutput=False)
    sink = nc.declare_dram_parameter("sink", [N, D], BF16, isOutput=False)
    outp = nc.declare_dram_parameter("outp", [N, C], BF16, isOutput=True)

    with tile.TileContext(nc) as tc:
        import contextlib

        with contextlib.ExitStack() as octx:
            # pools that live across stages
            persist = octx.enter_context(tc.tile_pool(name="persist", bufs=1))
            v_sb = [persist.tile([128, CS], BF16, name=f"v{i}") for i in range(NT)]
            # qT per (head, q-chunk): [d, 512 tokens]; split so attention on
            # q-chunk cq only depends on n-tiles 4cq..4cq+3 (lets attention
            # start before the whole of stage A finishes)
            qT_q = [[persist.tile([128, NQC], BF16, name=f"qT{h}_{c}")
                     for c in range(N // NQC)] for h in range(HPC)]
            # kT packed nt-major in two k-halves: block nt holds 4 heads x 128
            kT_pack = [persist.tile([128, N * HPC // 2], BF16, name=f"kTp{j}")
                       for j in range(2)]
            ident = persist.tile([128, 128], BF16, name="ident")
            make_identity(nc, ident[:])
            eps_sb = persist.tile([128, 1], F32, name="eps_sb")
            nc.vector.memset(eps_sb[:], EPS)
            # cos/sin planes preloaded whole so the rope chain never waits on
            # straggler tile DMAs behind the bulk weight stream
            cs_pl = {k: persist.tile([128, N], BF16, name=f"cs_{k}")
                     for k in ("cq", "sq", "ck", "sk")}

            # ---------------- Stage A: QKV + rmsnorm + rope + transpose ----
            with contextlib.ExitStack() as actx:
                p_wt = actx.enter_context(tc.tile_pool(name="p_wt", bufs=1))
                p_xt = actx.enter_context(tc.tile_pool(name="p_xt", bufs=2))
                p_ps = actx.enter_context(tc.tile_pool(name="p_ps", bufs=2, space="PSUM"))
                p_pst = actx.enter_context(tc.tile_pool(name="p_pst", bufs=1, space="PSUM"))
                p_sc = actx.enter_context(tc.tile_pool(name="p_sc", bufs=2))
                p_ro = actx.enter_context(tc.tile_pool(name="p_ro", bufs=2))
                p_ev = actx.enter_context(tc.tile_pool(name="p_ev", bufs=2))
                p_wm = actx.enter_context(tc.tile_pool(name="p_wm", bufs=1, space="PSUM"))
                warm_ps = p_wm.tile([128, 512], F32, name="warm_ps")

                def pe_keepalive(n):
                    # dependency-free dummy matmuls that keep the PE busy (and
                    # the HAM clock-gate open) while the stage-A tail drains
                    for _ in range(n):
                        nc.tensor.matmul(warm_ps[:], ident[:], v_sb[0][:],
                                         start=True, stop=True)

                # separate q/k/v weight tiles: finer DMA granularity so the
                # first matmuls' operands arrive quickly
                wt_sb = [[p_wt.tile([128, CS], BF16, name=f"wt{i}_{t}")
                          for t in range(3)] for i in range(CT)]
                xt0_sb = [p_xt.tile([128, NCHUNK], BF16, name=f"xt{i}") for i in range(CT)]
                for i in range(CT):
                    nc.sync.dma_start(out=xt0_sb[i][:], in_=xT[i * 128:(i + 1) * 128, 0:NCHUNK])
                    for t in range(3):
                        nc.sync.dma_start(out=wt_sb[i][t][:],
                                          in_=wT[i * 128:(i + 1) * 128, t * CS:(t + 1) * CS])
                    if i == 0:
                        # cos/sin right after the first weight triple
                        for nt2 in range(NT):
                            s2 = slice(nt2 * 128, (nt2 + 1) * 128)
                            nc.sync.dma_start(out=cs_pl["cq"][:, s2], in_=cosq[s2, :])
                            nc.sync.dma_start(out=cs_pl["sq"][:, s2], in_=sinq[s2, :])
                            nc.sync.dma_start(out=cs_pl["ck"][:, s2], in_=cosk[s2, :])
                            nc.sync.dma_start(out=cs_pl["sk"][:, s2], in_=sink[s2, :])

                for ch in range(N // NCHUNK):
                    n0 = ch * NCHUNK
                    if ch == 0:
                        xt_sb = xt0_sb
                    else:
                        xt_sb = [p_xt.tile([128, NCHUNK], BF16, name=f"xt{i}") for i in range(CT)]
                        for i in range(CT):
                            nc.sync.dma_start(
                                out=xt_sb[i][:],
                                in_=xT[i * 128:(i + 1) * 128, n0:n0 + NCHUNK])
                    for sub in range(NCHUNK // 128):
                        nt = (n0 + sub * 128) // 128
                        ps_q = p_ps.tile([128, CS], F32, name="ps_q")
                        ps_k = p_ps.tile([128, CS], F32, name="ps_k")
                        ps_v = p_ps.tile([128, CS], F32, name="ps_v")
                        for ci in range(CT):
                            st, sp = (ci == 0), (ci == CT - 1)
                            lhs = xt_sb[ci][:, sub * 128:(sub + 1) * 128]
                            nc.tensor.matmul(ps_q[:], lhs, wt_sb[ci][0][:],
                                             start=st, stop=sp)
                            nc.tensor.matmul(ps_k[:], lhs, wt_sb[ci][1][:],
                                             start=st, stop=sp)
                            nc.tensor.matmul(ps_v[:], lhs, wt_sb[ci][2][:],
                                             start=st, stop=sp)

                        if nt == NT - 1:
                            pe_keepalive(12)
                        # rmsnorm stats from the f32 psums (pre-quantization):
                        # ACT Square + accum_out -> per-row sum of squares
                        stats = p_sc.tile([128, 8], F32, name="stats")
                        dump = p_sc.tile([128, 128], F32, name="dump")
                        for t, ps in ((0, ps_q), (1, ps_k)):
                            for hl in range(HPC):
                                nc.scalar.activation(
                                    dump[:], ps[:, hl * D:(hl + 1) * D], AF.Square,
                                    accum_out=stats[:, t * 4 + hl:t * 4 + hl + 1])
                        # rstat = 1/sqrt(sumsq/D + eps)
                        rstat = p_sc.tile([128, 8], F32, name="rstat")
                        nc.scalar.activation(rstat[:], stats[:], AF.Sqrt,
                                             bias=eps_sb[:], scale=1.0 / D)
                        nc.vector.reciprocal(rstat[:], rstat[:])

                        # evac q,k to bf16 SBUF (ACT), v to persistent SBUF (DVE)
                        q_sb = p_ev.tile([128, CS], BF16, name="q_sb")
                        k_sb = p_ev.tile([128, CS], BF16, name="k_sb")
                        nc.scalar.copy(q_sb[:], ps_q[:])
                        nc.scalar.copy(k_sb[:], ps_k[:])
                        nc.vector.tensor_copy(v_sb[nt][:], ps_v[:])

                        # rope per head-slice (bf16 DVE), then transpose
                        psT = p_pst.tile([128, 1024], BF16, name="psT")
                        n0t = nt * 128
                        for t, ps, cpl, spl in (
                                (0, q_sb, cs_pl["cq"], cs_pl["sq"]),
                                (1, k_sb, cs_pl["ck"], cs_pl["sk"])):
                            for hl in range(HPC):
                                hsl = slice(hl * D, (hl + 1) * D)
                                r = rstat[:, t * 4 + hl:t * 4 + hl + 1]
                                tc_t = p_ro.tile([128, D], BF16, name="tc_t")
                                ts_t = p_ro.tile([128, D], BF16, name="ts_t")
                                ro_t = p_ro.tile([128, D], BF16, name="ro_t")
                                nc.vector.scalar_tensor_tensor(
                                    out=tc_t[:], in0=ps[:, hsl], scalar=r,
                                    in1=cpl[:, n0t:n0t + D],
                                    op0=ALU.mult, op1=ALU.mult)
                                nc.vector.scalar_tensor_tensor(
                                    out=ts_t[:, 0:64],
                                    in0=ps[:, hl * D + 64:hl * D + 128], scalar=r,
                                    in1=spl[:, n0t:n0t + 64],
                                    op0=ALU.mult, op1=ALU.mult)
                                nc.vector.scalar_tensor_tensor(
                                    out=ts_t[:, 64:128],
                                    in0=ps[:, hl * D:hl * D + 64], scalar=r,
                                    in1=spl[:, n0t + 64:n0t + 128],
                                    op0=ALU.mult, op1=ALU.mult)
                                nc.vector.tensor_add(ro_t[:], tc_t[:], ts_t[:])
                                nc.tensor.transpose(
                                    psT[:, t * 512 + hl * D:t * 512 + (hl + 1) * D],
                                    ro_t[:], ident[:])
                        # evac transposed q (4 per-head copies) + k (1 packed copy)
                        for hl in range(HPC):
                            nc.scalar.copy(
                                qT_q[hl][nt // 4][:, (nt % 4) * 128:(nt % 4 + 1) * 128],
                                psT[:, hl * D:(hl + 1) * D])
                        nc.scalar.copy(
                            kT_pack[nt // 8][:, (nt % 8) * 512:(nt % 8 + 1) * 512],
                            psT[:, 512:1024])
                        if nt == NT - 1:
                            pe_keepalive(10)

            # ---------------- Stage B+C fused: attention + projection -------
            p_bc = octx.enter_context(tc.tile_pool(name="p_bc", bufs=1))
            outT = [p_bc.tile([128, N], BF16, name=f"outT{h}") for h in range(HPC)]
            pwT_sb = [p_bc.tile([128, C], BF16, name=f"pw{h}") for h in range(HPC)]
            for h in range(HPC):
                nc.sync.dma_start(out=pwT_sb[h][:], in_=pwT[h * 128:(h + 1) * 128, :])

            with contextlib.ExitStack() as bctx:
                p_sT = bctx.enter_context(tc.tile_pool(name="p_sT", bufs=2, space="PSUM"))
                p_pv = bctx.enter_context(tc.tile_pool(name="p_pv", bufs=1, space="PSUM"))
                p_sm = bctx.enter_context(tc.tile_pool(name="p_sm", bufs=1, space="PSUM"))
                p_pc = bctx.enter_context(tc.tile_pool(name="p_pc", bufs=2, space="PSUM"))
                p_pt = bctx.enter_context(tc.tile_pool(name="p_pt", bufs=4))
                p_ac = bctx.enter_context(tc.tile_pool(name="p_ac", bufs=3))
                p_fo = bctx.enter_context(tc.tile_pool(name="p_fo", bufs=4))
                ones_sb = p_bc.tile([128, 1], BF16, name="ones_sb")
                nc.vector.memset(ones_sb[:], 1.0)
                ones_row = p_bc.tile([1, 128], F32, name="ones_row")
                nc.vector.memset(ones_row[:], 1.0)

                # denominator work split: DVE accumulates exp tiles for nkp
                # 0..NPE-1 into acc2; the PE reduces the last pairs (and acc2)
                # with ones-matmuls into one [1,512] PSUM accumulator.
                NPE = 7  # nkp handled by DVE accumulation

                def attn_block(h, cq):
                    pv_ps = p_pv.tile([128, NQC], F32, name="pv_ps")
                    sm_ps = p_sm.tile([1, NQC], F32, name="sm_ps")
                    acc2 = p_ac.tile([128, 1024], BF16, name="acc2")
                    for nkp in range(NT // 2):
                        sT2 = p_sT.tile([128, 1024], F32, name="sT2")
                        pt2 = p_pt.tile([128, 1024], BF16, name="pt2")
                        for half in range(2):
                            nk = nkp * 2 + half
                            ksl = slice((nk % 8) * 512 + h * 128,
                                        (nk % 8) * 512 + (h + 1) * 128)
                            nc.tensor.matmul(sT2[:, half * 512:(half + 1) * 512],
                                             kT_pack[nk // 8][:, ksl],
                                             qT_q[h][cq][:], start=True, stop=True)
                        nc.scalar.activation(pt2[:], sT2[:], AF.Exp)
                        for half in range(2):
                            nk = nkp * 2 + half
                            fsl = slice(half * 512, (half + 1) * 512)
                            nc.tensor.matmul(pv_ps[:],
                                             v_sb[nk][:, h * D:(h + 1) * D],
                                             pt2[:, fsl], start=(nk == 0),
                                             stop=(nk == NT - 1))
                        if nkp < NPE:
                            if nkp == 0:
                                nc.vector.tensor_copy(acc2[:], pt2[:])
                            else:
                                nc.vector.tensor_add(acc2[:], acc2[:], pt2[:])
                        else:
                            for half in range(2):
                                fsl = slice(half * 512, (half + 1) * 512)
                                nc.tensor.matmul(sm_ps[:], ones_sb[:], pt2[:, fsl],
                                                 start=(nkp == NPE and half == 0),
                                                 stop=False)
                    nc.tensor.matmul(sm_ps[:], ones_sb[:], acc2[:, 0:512],
                                     start=(NPE == NT // 2), stop=False)
                    nc.tensor.matmul(sm_ps[:], ones_sb[:], acc2[:, 512:1024],
                                     start=False, stop=True)

                    # free the pv bank quickly, normalize from SBUF later
                    pv_sb = p_ac.tile([128, NQC], F32, name="pv_sb")
                    nc.vector.tensor_copy(pv_sb[:], pv_ps[:])
                    recip = p_ac.tile([1, NQC], F32, name="recip")
                    nc.vector.reciprocal_approx_fast(out=recip[:], in_=sm_ps[0:1, :])
                    # broadcast 1/denom across partitions as a K=1 outer
                    # product on the PE (gpsimd dispatch is multi-us flaky);
                    # fp16 keeps the matmul at 1 cycle/row with a 10-bit mantissa
                    bc_ps = p_sm.tile([128, NQC], F32, name="sm_ps")
                    nc.tensor.matmul(bc_ps[:], ones_row[:], recip[:],
                                     start=True, stop=True)
                    nc.vector.tensor_mul(outT[h][:, cq * NQC:(cq + 1) * NQC],
                                         pv_sb[:], bc_ps[:])

                def proj_block(cq):
                    for nt in range(cq * 4, cq * 4 + 4):
                        for oc in range(C // 512):
                            ps_c = p_pc.tile([128, 512], F32, name="ps_c")
                            for h in range(HPC):
                                nc.tensor.matmul(ps_c[:],
                                                 outT[h][:, nt * 128:(nt + 1) * 128],
                                                 pwT_sb[h][:, oc * 512:(oc + 1) * 512],
                                                 start=(h == 0), stop=(h == HPC - 1))
                            fo_t = p_fo.tile([128, 512], BF16, name="fo_t")
                            nc.vector.tensor_copy(fo_t[:], ps_c[:])
                            nc.sync.dma_start(
                                out=outp[nt * 128:(nt + 1) * 128, oc * 512:(oc + 1) * 512],
                                in_=fo_t[:])

                # software-pipelined: proj for block cq-1 is emitted after the
                # attention of block cq, so proj matmuls never wait on the
                # normalize chain tail. The cq0->cq1 boundary has no proj work
                # to bridge the chain latency, so keep the PE warm with
                # dependency-free dummies into a (still unused) proj bank.
                for cq in range(N // NQC + 1):
                    if cq < N // NQC:
                        for h in range(HPC):
                            attn_block(h, cq)
                    if cq == 0:
                        bridge = p_sT.tile([128, 1024], F32, name="sT2")
                        for _ in range(14):
                            nc.tensor.matmul(bridge[:, 0:512], ident[:], v_sb[0][:],
                                             start=True, stop=True)
                    if cq > 0:
                        proj_block(cq - 1)

    nc.finalize()
    return nc


def make_in_maps(x, rope_cos, rope_sin, qkv_w, proj_w, q_norm_w, k_norm_w):
    import ml_dtypes
    bf16 = ml_dtypes.bfloat16
    scale = np.float32(D ** -0.5)

    def fold(w, scaled):
        cos = rope_cos * w[None, :]
        sf = np.empty_like(rope_sin)
        sf[:, :64] = -rope_sin[:, :64] * w[None, 64:]
        sf[:, 64:] = rope_sin[:, 64:] * w[None, :64]
        if scaled:
            cos = cos * scale
            sf = sf * scale
        return (np.ascontiguousarray(cos).astype(bf16),
                np.ascontiguousarray(sf).astype(bf16))

    cosq, sinq = fold(q_norm_w, True)
    cosk, sink = fold(k_norm_w, False)

    in_maps = []
    for core in range(NCORES):
        b, hg = core // 4, core % 4
        c0 = hg * CS
        rows = np.concatenate([
            qkv_w[c0:c0 + CS], qkv_w[C + c0:C + c0 + CS],
            qkv_w[2 * C + c0:2 * C + c0 + CS]], axis=0)
        in_maps.append({
            "xT": np.ascontiguousarray(x[b].T).astype(bf16),
            "wT": np.ascontiguousarray(rows.T).astype(bf16),
            "pwT": np.ascontiguousarray(proj_w[:, c0:c0 + CS].T).astype(bf16),
            "cosq": cosq, "sinq": sinq, "cosk": cosk, "sink": sink,
        })
    return in_maps


def gather(results, proj_b):
    out = np.empty((B, N, C), np.float32)
    for b in range(B):
        acc = np.zeros((N, C), np.float64)
        for hg in range(4):
            acc += np.asarray(results[b * 4 + hg]["outp"]).astype(np.float64)
        out[b] = (acc + proj_b.astype(np.float64)[None, :]).astype(np.float32)
    return out


LAST_RESULTS = None  # BassKernelResults of the most recent kernel() call


def kernel(x, rope_cos, rope_sin, qkv_w, proj_w, proj_b, q_norm_w, k_norm_w):
    import os
    from concourse.bass_utils import run_bass_kernel_spmd

    global LAST_RESULTS
    x = np.asarray(x, np.float32)
    in_maps = make_in_maps(np.asarray(x, np.float32), np.asarray(rope_cos, np.float32),
                           np.asarray(rope_sin, np.float32), np.asarray(qkv_w, np.float32),
                           np.asarray(proj_w, np.float32), np.asarray(q_norm_w, np.float32),
                           np.asarray(k_norm_w, np.float32))
    nc = build_nc()
    trace = bool(os.environ.get("BASS_KERNEL_TRACE"))
    try:
        res = run_bass_kernel_spmd(nc, in_maps, list(range(NCORES)), trace=trace)
    except Exception:
        # transient device wedge (e.g. NRT_EXEC_UNIT_UNRECOVERABLE) — retry once
        res = run_bass_kernel_spmd(build_nc(), in_maps, list(range(NCORES)), trace=trace)
    LAST_RESULTS = res
    return gather(res.results, np.asarray(proj_b, np.float32))


if __name__ == "__main__":
    rng = np.random.default_rng(0)
    out = kernel(
        x=rng.standard_normal((B, N, C)).astype(np.float32),
        rope_cos=rng.random((N, D), dtype=np.float32),
        rope_sin=rng.random((N, D), dtype=np.float32),
        qkv_w=(rng.standard_normal((3 * C, C)) * C ** -0.5).astype(np.float32),
        proj_w=(rng.standard_normal((C, C)) * C ** -0.5).astype(np.float32),
        proj_b=np.zeros((C,), np.float32),
        q_norm_w=np.ones((D,), np.float32),
        k_norm_w=np.ones((D,), np.float32),
    )
    print(out.shape, out.dtype)
